# revision 56
# baseline (speedup 1.0000x reference)
"""Trainium2 Bass kernel for nn_Block_19069654794616 (dense transformer block).

B=2, S=2048, D=1600, 25 heads x 64, causal attention, 4x MLP (tanh-gelu),
pre-LN with residuals. fp32 in/out.

Distribution (8 NeuronCores, token-parallel):
  Core j owns 512 tokens: chunk A = seq0[256j:256j+256], chunk B =
  seq1[256(7-j):256(8-j)] (mirrored pairing balances the causal triangle:
  every core has exactly 9 valid 256-token key super-chunks).
  - LN1 + QKV computed per-core on its own tokens (LN gains/biases folded
    into the QKV weights host-side; q pre-scaled by c^-0.5).
  - All GEMMs run in bf16 (fp32 PSUM accumulation); the k/v shards are
    AllGather'd in bf16 (one collective, half the bytes of fp32).
  - The 2 diagonal key super-chunks use the core's OWN k/v straight from
    SBUF, so they run while the AllGather is still in flight; the causal
    triangle is added via one static identity-matmul mask.
  - The up-to-7-per-chunk sub-diagonal super-chunks are fully valid (no
    mask at all) and are emitted as 14 static slots guarded by tc.If on
    host-supplied validity flags: each core executes exactly its 7.
  - Softmax denominators come from a ones-column appended to v (row 64 of
    the 65-row AV output); normalization happens once per head at the end.
  - proj / LN2 / MLP are token-local; outputs concatenated host-side.
"""

import numpy as np

import concourse.bass as bass
import concourse.mybir as mybir
import concourse.tile as tile
from concourse import bacc
from concourse.bass_utils import run_bass_kernel_spmd
from concourse.masks import make_identity

f32 = mybir.dt.float32
bf16 = mybir.dt.bfloat16
i32 = mybir.dt.int32
BF16NP = mybir.dt.np(mybir.dt.bfloat16)

N_CORES = 8
B, S, D = 2, 2048, 1600
H, C = 25, 64
D3, D4 = 3 * D, 4 * D
TOK = 512          # tokens per core
CH = 256           # query chunk (2 per core)
LC = 128           # L-chunk (matmul partition tile)
EPS = 1e-5
NHP = 13           # head-pair tiles (12 pairs + head 24)
NDS = 7            # static non-diagonal slots per chunk (<=7 taken of 14)

# D contraction chunks: 12x128 + 1x64
DCH = [(t * 128, 128) for t in range(12)] + [(1536, 64)]
# output-column tiles of 400 for D-sized outputs
NJ = [(j * 400, 400) for j in range(4)]

KREG = NHP * 128 * TOK          # elements in the k^T region of a shard
VREG = 4 * 128 * D              # elements in the v region
SHARD = KREG + VREG

IF_ENGINES = (mybir.EngineType.SP, mybir.EngineType.PE,
              mybir.EngineType.Activation, mybir.EngineType.DVE)


def _build():
    nc = bacc.Bacc(
        "TRN2",
        target_bir_lowering=False,
        debug=False,
        enable_asserts=True,
        num_devices=N_CORES,
    )
    x_in = nc.dram_tensor("x", [TOK, D], f32, kind="ExternalInput").ap()
    wqkv = nc.dram_tensor("wqkv", [D, D3], bf16, kind="ExternalInput").ap()
    bqkv = nc.dram_tensor("bqkv", [D3], f32, kind="ExternalInput").ap()
    wproj = nc.dram_tensor("wproj", [D, D], bf16, kind="ExternalInput").ap()
    bproj = nc.dram_tensor("bproj", [D], f32, kind="ExternalInput").ap()
    wfc = nc.dram_tensor("wfc", [D, D4], bf16, kind="ExternalInput").ap()
    bfc = nc.dram_tensor("bfc", [D4], f32, kind="ExternalInput").ap()
    wout = nc.dram_tensor("wout", [D4, D], bf16, kind="ExternalInput").ap()
    bout = nc.dram_tensor("bout", [D], f32, kind="ExternalInput").ap()
    trimask = nc.dram_tensor("trimask", [128, 2, CH], bf16,
                             kind="ExternalInput").ap()
    flags = nc.dram_tensor("flags", [1, 2 * NDS], i32,
                           kind="ExternalInput").ap()
    out = nc.dram_tensor("out", [TOK, D], f32, kind="ExternalOutput").ap()

    shard = nc.dram_tensor("shard", [1, SHARD], bf16, kind="Internal").ap()
    kv_all = nc.dram_tensor(
        "kv_all", [N_CORES, SHARD], bf16, kind="Internal", addr_space="Shared"
    ).ap()

    with tile.TileContext(nc, pool_alloc_mode="queue") as tc:
        _emit(tc, nc, x_in, wqkv, bqkv, wproj, bproj, wfc, bfc, wout, bout,
              trimask, flags, out, shard, kv_all)
    nc.compile()
    return nc


def _emit(tc, nc, x_in, wqkv, bqkv, wproj, bproj, wfc, bfc, wout, bout,
          trimask, flags, out, shard, kv_all):
    sync, vec, act, gp, te = nc.sync, nc.vector, nc.scalar, nc.gpsimd, nc.tensor
    AluOp = mybir.AluOpType
    Act = mybir.ActivationFunctionType

    # ---------------- whole-kernel pools ----------------
    psB = tc.alloc_tile_pool(name="psB", bufs=4, space="PSUM")  # qkv-era 1-bank
    persist = tc.alloc_tile_pool(name="persist", bufs=1)

    ident = persist.tile([128, 128], f32, name="ident")
    make_identity(nc, ident)
    ident_b = persist.tile([128, 128], bf16, name="ident_b")
    vec.tensor_copy(out=ident_b, in_=ident)
    eps_t = persist.tile([128, 1], f32, name="eps_t")
    vec.memset(eps_t, EPS)
    warm_t = persist.tile([1, 1], f32, name="warm_t")

    def warm_sqrt_table():
        # pull the Sqrt act-table load off the next LN chain: run a dummy
        # Sqrt while the Act engine is otherwise idle
        act.activation(out=warm_t, in_=eps_t[0:1, 0:1], func=Act.Sqrt)

    warm_sqrt_table()   # overlaps the initial x DMA, feeds LN1's first rstd
    flags_sb = persist.tile([1, 2 * NDS], i32, name="flags_sb")
    tri_sb = persist.tile([128, 2, CH], bf16, name="tri_sb")

    # per-partition bias columns for k / q / fc (out-channel on partitions)
    bk_col = persist.tile([128, NHP], f32, name="bk_col")
    bq_col = persist.tile([128, NHP], f32, name="bq_col")
    bfc_col = persist.tile([128, 50], f32, name="bfc_col")

    def load_persist_smalls():
        # deferred until after the x loads: nothing here is needed before
        # the q/k bias adds, and they'd delay LN1's critical-path DMAs
        sync.dma_start(out=flags_sb, in_=flags)
        sync.dma_start(out=tri_sb, in_=trimask)
        sync.dma_start(out=bq_col, in_=bass.AP(
            tensor=bqkv.tensor, offset=0, ap=[[1, 128], [128, NHP]]))
        sync.dma_start(out=bk_col, in_=bass.AP(
            tensor=bqkv.tensor, offset=D, ap=[[1, 128], [128, NHP]]))
        sync.dma_start(out=bfc_col, in_=bass.AP(
            tensor=bfc.tensor, offset=0, ap=[[1, 128], [128, 50]]))
    # replicated (per-free) bias rows (DMAs deferred until after k^T)
    bv_rep = persist.tile([128, D], f32, name="bv_rep")
    bproj_rep = persist.tile([128, D], f32, name="bproj_rep")
    bout_rep = persist.tile([128, D], f32, name="bout_rep")

    def load_bv_rep():
        sync.dma_start(out=bv_rep, in_=bass.AP(
            tensor=bqkv.tensor, offset=2 * D, ap=[[0, 128], [1, D]]))

    def load_bias_reps():
        sync.dma_start(out=bproj_rep, in_=bass.AP(
            tensor=bproj.tensor, offset=0, ap=[[0, 128], [1, D]]))
        sync.dma_start(out=bout_rep, in_=bass.AP(
            tensor=bout.tensor, offset=0, ap=[[0, 128], [1, D]]))

    def load_w_big(pool, src, col0, name):
        """(1600, 128-col) bf16 weight slice -> (128, 13, 128) tile."""
        w = pool.tile([128, NHP, 128], bf16, name=name, tag="wbig", bufs=6)
        src_t = src.tensor
        ncols = src.shape[1]
        sync.dma_start(out=w[:, 0:12, :], in_=bass.AP(
            tensor=src_t, offset=col0,
            ap=[[ncols, 128], [128 * ncols, 12], [1, 128]]))
        sync.dma_start(out=w[:64, 12:13, :], in_=bass.AP(
            tensor=src_t, offset=1536 * ncols + col0,
            ap=[[ncols, 64], [128 * ncols, 1], [1, 128]]))
        return w

    def ln_transpose(get_src, dst_tiles, pool, label, pspool=None):
        """get_src(tt) -> (128, D) fp32 SBUF tile; LN + transpose into
        13 (128, TOK) bf16 dst tiles."""
        for tt in range(4):
            xt = get_src(tt)
            stats = pool.tile([128, 4, 6], f32, name=f"{label}st{tt}",
                              tag=f"{label}st")
            xg = xt.rearrange("p (g d) -> p g d", g=4)
            for g in range(4):
                vec.bn_stats(out=stats[:, g, :], in_=xg[:, g, :])
            mv = pool.tile([128, 2], f32, name=f"{label}mv{tt}", tag=f"{label}mv")
            vec.bn_aggr(out=mv, in_=stats)
            rstd = pool.tile([128, 1], f32, name=f"{label}rs{tt}",
                             tag=f"{label}rs")
            act.activation(out=rstd, in_=mv[:, 1:2], func=Act.Sqrt, bias=eps_t)
            vec.reciprocal(out=rstd, in_=rstd)
            # xc in bf16: the transposed destination is bf16 anyway, and a
            # bf16 transpose runs 1 cycle/row on the PE vs 2 for fp32
            xc = pool.tile([128, D], bf16, name=f"{label}xc{tt}",
                           tag=f"{label}xc")
            # split so the first transposes start before the full row is done
            vec.tensor_scalar(out=xc[:, 0:512], in0=xt[:, 0:512],
                              scalar1=mv[:, 0:1], scalar2=rstd,
                              op0=AluOp.subtract, op1=AluOp.mult)
            vec.tensor_scalar(out=xc[:, 512:D], in0=xt[:, 512:D],
                              scalar1=mv[:, 0:1], scalar2=rstd,
                              op0=AluOp.subtract, op1=AluOp.mult)
            for t, (d0, dp) in enumerate(DCH):
                tp = (pspool or psB).tile([128, 128], bf16, name=f"{label}tp",
                                          tag="ps1")
                te.transpose(tp[:dp, :], xc[:, d0:d0 + dp], ident_b)
                vec.tensor_copy(out=dst_tiles[t][:dp, tt * 128:(tt + 1) * 128],
                                in_=tp[:dp, :])

    # ---- long-lived pools, allocated outermost-first (stack discipline) ----
    pool_y = tc.alloc_tile_pool(name="pool_y", bufs=1)      # residual stream
    y = [pool_y.tile([128, D], f32, name=f"y{tt}", tag=f"y{tt}")
         for tt in range(4)]
    pool_qT = tc.alloc_tile_pool(name="pool_qT", bufs=1)    # until proj
    qT = [pool_qT.tile([128, TOK], bf16, name=f"qT{t}", tag=f"qT{t}")
          for t in range(NHP)]
    pool_at = tc.alloc_tile_pool(name="pool_at", bufs=1)    # until proj
    attn_T = [pool_at.tile([128, TOK], bf16, name=f"attnT{t}", tag=f"attnT{t}")
              for t in range(NHP)]
    # proj weight pool allocated early so its DMAs prefetch during attention
    pool_p10 = tc.alloc_tile_pool(name="pool_p10", bufs=3)
    pool_ptm = tc.alloc_tile_pool(name="pool_ptm", bufs=3)  # whole attention
    pool_acc = tc.alloc_tile_pool(name="pool_acc", bufs=1)
    acc = [pool_acc.tile([65, H, CH], f32, name=f"acc{c}") for c in range(2)]

    # ======== S1: LN1 -> xcT; k,v; shard; AllGather; q; diag attn ========
    pool_s1 = tc.alloc_tile_pool(name="pool_s1", bufs=1)
    kT = [pool_s1.tile([128, TOK], bf16, name=f"kT{t}", tag=f"kT{t}")
          for t in range(NHP)]
    vown = [pool_s1.tile([128, D], bf16, name=f"vown{tt}", tag=f"vown{tt}")
            for tt in range(4)]
    pool_xc = tc.alloc_tile_pool(name="pool_xc", bufs=1)
    xcT = [pool_xc.tile([128, TOK], bf16, name=f"xcT{t}", tag=f"xcT{t}")
           for t in range(NHP)]

    pool_ln = tc.alloc_tile_pool(name="pool_ln", bufs=2)

    def ln1_src(tt):
        xt = pool_ln.tile([128, D], f32, name=f"ln1x{tt}", tag="ln1x")
        # two pieces: the first two bn_stats groups start ~1.5us earlier
        sync.dma_start(out=xt[:, 0:800],
                       in_=x_in[tt * 128:(tt + 1) * 128, 0:800])
        sync.dma_start(out=xt[:, 800:D],
                       in_=x_in[tt * 128:(tt + 1) * 128, 800:D])
        return xt

    ln_transpose(ln1_src, xcT, pool_ln, "ln1")
    pool_ln.release()

    pool_w1 = tc.alloc_tile_pool(name="pool_w1", bufs=3)

    def qk_proj(col_base, bias_col, dst):
        """dst[t] (128, TOK) = (w[:, cols].T @ xc^T) + bias."""
        for t in range(NHP):
            w = load_w_big(pool_w1, wqkv, col_base + t * 128, f"w{col_base}_{t}")
            ps = psB.tile([128, TOK], f32, name="qkps", tag="ps1")
            for ci, (d0, dp) in enumerate(DCH):
                te.matmul(ps, lhsT=w[:dp, ci, :], rhs=xcT[ci][:dp, :],
                          start=(ci == 0), stop=(ci == len(DCH) - 1))
            vec.tensor_scalar(out=dst[t], in0=ps, scalar1=bias_col[:, t:t + 1],
                              scalar2=None, op0=AluOp.add)

    kreg = shard[0, 0:KREG].rearrange("(t p n) -> t p n", t=NHP, p=128)
    vreg = shard[0, KREG:SHARD].rearrange("(t p n) -> t p n", t=4, p=128)

    # v first: its (ci=0, tt=0) matmul only needs the first transposed tile,
    # so PE starts ~10us earlier and the AllGather gate moves up
    vps = {}
    for j0, (c0, cw) in enumerate(NJ):
        for ci, (d0, dp) in enumerate(DCH):
            wv = pool_w1.tile([128, 400], bf16, name=f"wv{j0}_{ci}",
                              tag="w400", bufs=6)
            sync.dma_start(out=wv[:dp, :], in_=bass.AP(
                tensor=wqkv.tensor, offset=d0 * D3 + 2 * D + c0,
                ap=[[D3, dp], [1, cw]]))
            if j0 == 0 and ci == 6:
                load_bv_rep()
            for tt in range(4):
                if ci == 0:
                    vps[tt] = psB.tile([128, 400], f32, name=f"vps{tt}",
                                       tag="ps1")
                te.matmul(vps[tt], lhsT=xcT[ci][:dp, tt * 128:(tt + 1) * 128],
                          rhs=wv[:dp, :], start=(ci == 0),
                          stop=(ci == len(DCH) - 1))
        for tt in range(4):
            vec.tensor_tensor(out=vown[tt][:, c0:c0 + cw], in0=vps[tt],
                              in1=bv_rep[:, c0:c0 + cw], op=AluOp.add)
    for tt in range(4):
        # shard writes on the act queue: they wait on compute sems and would
        # otherwise stall the in-order sync queue's weight prefetches
        act.dma_start(out=vreg[tt], in_=vown[tt])
    load_persist_smalls()

    qk_proj(D, bk_col, kT)          # k^T
    for t in range(NHP):
        act.dma_start(out=kreg[t], in_=kT[t])
    load_bias_reps()

    gp.collective_compute(
        "AllGather", mybir.AluOpType.bypass,
        replica_groups=[list(range(N_CORES))],
        ins=[shard], outs=[kv_all],
    )

    qk_proj(0, bq_col, qT)          # q^T (pre-scaled by c^-0.5 host-side)
    pool_w1.release()
    pool_xc.release()

    # ======== attention ========
    psB.release()
    psA = tc.alloc_tile_pool(name="psA", bufs=3, space="PSUM")   # st4: 2 banks
    psAv = tc.alloc_tile_pool(name="psAv", bufs=2, space="PSUM")  # av: 1 bank

    def slot_compute(c, kt_ap, vt_ap, first, masked):
        """One 256-key super-chunk against query chunk c.
        kt_ap(hp) -> [128 cdims, CH keys]; vt_ap(lc, h) -> [128 keys, C+1]."""
        for hp in range(NHP):
            nh = 1 if hp == 12 else 2
            st = psA.tile([128, 4, CH], f32, name="st", tag="st4")
            for hh in range(nh):
                p0 = hh * 64
                for lc in range(2):
                    te.matmul(st[:, hh * 2 + lc, :],
                              lhsT=kt_ap(hp, p0, lc),
                              rhs=qT[hp][p0:p0 + 64, c * CH:(c + 1) * CH],
                              start=(lc == 0),
                              stop=(not masked and lc == 1))
            if masked:
                # within-chunk causal triangle: st[:, (hh,lc), :] += tri[lc]
                for hh in range(nh):
                    for lc in range(2):
                        te.matmul(st[:, hh * 2 + lc, :], lhsT=ident_b,
                                  rhs=tri_sb[:, lc, :], start=False,
                                  stop=(lc == 1))
            ptm = pool_ptm.tile([128, 4, CH], bf16, name="ptm", tag="ptm")
            act.activation(out=ptm[:, 0:2 * nh, :], in_=st[:, 0:2 * nh, :],
                           func=Act.Exp)
            # both heads of the pair share one PSUM bank so the accumulate
            # into acc is a single DVE op per head-pair
            av2 = psAv.tile([65, 2, CH], f32, name="av2", tag="av")
            for hh in range(nh):
                h = hp * 2 + hh
                for lc in range(2):
                    te.matmul(av2[:, hh, :], lhsT=vt_ap(lc, h),
                              rhs=ptm[:, hh * 2 + lc, :],
                              start=(hh == 0 and lc == 0),
                              stop=(hh == nh - 1 and lc == 1))
            if first:
                vec.tensor_copy(out=acc[c][:, 2 * hp:2 * hp + nh, :],
                                in_=av2[:, 0:nh, :])
            else:
                vec.tensor_tensor(out=acc[c][:, 2 * hp:2 * hp + nh, :],
                                  in0=acc[c][:, 2 * hp:2 * hp + nh, :],
                                  in1=av2[:, 0:nh, :], op=AluOp.add)

    # ---- diagonal slots from LOCAL k/v (overlap the AllGather) ----
    pool_vtd = tc.alloc_tile_pool(name="pool_vtd", bufs=1)
    vtd = []
    for c in range(2):
        vt = pool_vtd.tile([128, 2, H, C + 1], bf16, name=f"vtd{c}")
        vec.memset(vt, 1.0)
        for lc in range(2):
            vsrc = vown[2 * c + lc].rearrange("p (h c) -> p h c", h=H)
            vec.tensor_copy(out=vt[:, lc, :, 0:C], in_=vsrc)
        vtd.append(vt)
    for c in range(2):
        slot_compute(
            c,
            kt_ap=lambda hp, p0, lc, c=c: kT[hp][
                p0:p0 + 64, c * CH + lc * LC:c * CH + (lc + 1) * LC],
            vt_ap=lambda lc, h, c=c: vtd[c][:, lc, h, :],
            first=True, masked=True)
    pool_vtd.release()
    pool_s1.release()

    # ---- sub-diagonal slots from the gathered k/v, If-guarded ----
    pool_sc = tc.alloc_tile_pool(name="pool_sc", bufs=2)
    pool_nrm = tc.alloc_tile_pool(name="pool_nrm", bufs=3)
    for c in range(2):
        for s in range(NDS):
            r = s if c == 0 else 7 - s
            toff = c * CH
            kt = pool_sc.tile([128, NHP, CH], bf16, name="kt", tag="kt", bufs=3)
            vt = pool_sc.tile([128, 2, H, C + 1], bf16, name="vt", tag="vt")
            # denominator ones-column (the slot DMAs only write [.., 0:C])
            vec.memset(vt[:, :, :, C:C + 1], 1.0)
            regs = [nc.alloc_register(e, f"fl{c}_{s}_{e.name}")
                    for e in IF_ENGINES]
            nc.regs_load(bass.RegisterHandles(regs),
                         flags_sb[0:1, c * NDS + s:c * NDS + s + 1])
            val = nc.snap(bass.RegisterHandles(regs), min_val=0, max_val=1)
            with tc.If(val != 0):
                sync.dma_start(out=kt, in_=bass.AP(
                    tensor=kv_all.tensor, offset=r * SHARD + toff,
                    ap=[[TOK, 128], [128 * TOK, NHP], [1, CH]]))
                for lc in range(2):
                    sync.dma_start(out=vt[:, lc, :, 0:C], in_=bass.AP(
                        tensor=kv_all.tensor,
                        offset=r * SHARD + KREG + (2 * c + lc) * 128 * D,
                        ap=[[D, 128], [C, H], [1, C]]))
                slot_compute(
                    c,
                    kt_ap=lambda hp, p0, lc, kt=kt: kt[
                        p0:p0 + 64, hp, lc * LC:(lc + 1) * LC],
                    vt_ap=lambda lc, h, vt=vt: vt[:, lc, h, :],
                    first=False, masked=False)
        # ---- normalize chunk c as soon as its slots are done ----
        for h in range(H):
            rcp = pool_nrm.tile([1, CH], f32, name="rcp", tag="rcp")
            vec.reciprocal(out=rcp, in_=acc[c][64:65, h, :])
            rcpb = pool_nrm.tile([64, CH], f32, name="rcpb", tag="rcpb")
            gp.partition_broadcast(rcpb, rcp)
            nrm = pool_nrm.tile([64, CH], bf16, name="nrm", tag="nrm")
            vec.tensor_tensor(out=nrm, in0=acc[c][0:64, h, :], in1=rcpb,
                              op=AluOp.mult)
            # act queue: these wait on the mult sems and would block the
            # in-order sync queue ahead of the proj weight prefetches
            act.dma_start(
                out=attn_T[h // 2][(h % 2) * 64:(h % 2) * 64 + 64,
                                   c * CH:(c + 1) * CH],
                in_=nrm)

    # the attention exps evicted the Sqrt table; reload it now while the Act
    # engine idles through proj, so LN2's first rstd doesn't pay for it
    warm_sqrt_table()

    pool_nrm.release()
    pool_sc.release()
    pool_acc.release()
    pool_ptm.release()

    psAv.release()
    psA.release()
    psC = tc.alloc_tile_pool(name="psC", bufs=4, space="PSUM")
    psD = tc.alloc_tile_pool(name="psD", bufs=4, space="PSUM")

    # ======== proj + residual -> y (SBUF-resident) ========
    # LN2 stats are computed incrementally as each 400-col block of y lands
    ln2_stats = [pool_y.tile([128, 4, 6], f32, name=f"l2s{tt}", tag=f"l2s{tt}")
                 for tt in range(4)]
    for j0, (c0, cw) in enumerate(NJ):
        # stage the column block's 13 weight tiles, then drive one PSUM tile
        # per tt so psC stays 4-deep pipelined across (cg, tt) pairs
        wps = []
        for ci, (d0, dp) in enumerate(DCH):
            wp = pool_p10.tile([128, 400], bf16, name=f"wp{j0}_{ci}",
                               tag="w400", bufs=14)
            sync.dma_start(out=wp[:dp, :], in_=bass.AP(
                tensor=wproj.tensor, offset=d0 * D + c0,
                ap=[[D, dp], [1, cw]]))
            wps.append(wp)
        for tt in range(4):
            pps = psC.tile([128, 400], f32, name="pps", tag="psc1")
            for ci, (d0, dp) in enumerate(DCH):
                te.matmul(pps, lhsT=attn_T[ci][:dp, tt * 128:(tt + 1) * 128],
                          rhs=wps[ci][:dp, :], start=(ci == 0),
                          stop=(ci == len(DCH) - 1))
            xr = pool_p10.tile([128, 400], f32, name=f"xr{tt}", tag="xr")
            sync.dma_start(out=xr, in_=x_in[tt * 128:(tt + 1) * 128, c0:c0 + cw])
            vec.tensor_tensor(out=y[tt][:, c0:c0 + cw], in0=pps,
                              in1=xr, op=AluOp.add)
            vec.tensor_tensor(out=y[tt][:, c0:c0 + cw],
                              in0=y[tt][:, c0:c0 + cw],
                              in1=bproj_rep[:, c0:c0 + cw], op=AluOp.add)
            vec.bn_stats(out=ln2_stats[tt][:, j0, :],
                         in_=y[tt][:, c0:c0 + cw])
    pool_p10.release()
    pool_at.release()
    pool_qT.release()

    # ======== LN2 -> ycT; MLP; out ========
    pool_s4 = tc.alloc_tile_pool(name="pool_s4", bufs=1)
    ycT = [pool_s4.tile([128, TOK], bf16, name=f"ycT{t}", tag=f"ycT{t}")
           for t in range(NHP)]
    pool_ln2 = tc.alloc_tile_pool(name="pool_ln2", bufs=2)
    for tt in range(4):
        mv = pool_ln2.tile([128, 2], f32, name=f"ln2mv{tt}", tag="ln2mv")
        vec.bn_aggr(out=mv, in_=ln2_stats[tt])
        rstd = pool_ln2.tile([128, 1], f32, name=f"ln2rs{tt}", tag="ln2rs")
        act.activation(out=rstd, in_=mv[:, 1:2], func=Act.Sqrt, bias=eps_t)
        vec.reciprocal(out=rstd, in_=rstd)
        xc = pool_ln2.tile([128, D], f32, name=f"ln2xc{tt}", tag="ln2xc")
        vec.tensor_scalar(out=xc, in0=y[tt], scalar1=mv[:, 0:1], scalar2=rstd,
                          op0=AluOp.subtract, op1=AluOp.mult)
        for t, (d0, dp) in enumerate(DCH):
            tp = psD.tile([128, 128], f32, name="ln2tp", tag="ps1")
            te.transpose(tp[:dp, :], xc[:, d0:d0 + dp], ident)
            vec.tensor_copy(out=ycT[t][:dp, tt * 128:(tt + 1) * 128],
                            in_=tp[:dp, :])
    pool_ln2.release()

    pool_h = tc.alloc_tile_pool(name="pool_h", bufs=2)
    pool_w2 = tc.alloc_tile_pool(name="pool_w2", bufs=3)
    GRP = [17, 17, 16]

    ops = {}
    f_base = 0
    for gi, ng in enumerate(GRP):
        last_grp = (gi == len(GRP) - 1)
        hT = [pool_h.tile([128, TOK], bf16, name=f"hT{f_base}_{fi}",
                          tag=f"hT{fi}") for fi in range(ng)]
        for fi in range(ng):
            f = f_base + fi
            wf = load_w_big(pool_w2, wfc, f * 128, f"wf{f}")
            ps = psD.tile([128, TOK], f32, name="hps", tag="ps1")
            for ci, (d0, dp) in enumerate(DCH):
                te.matmul(ps, lhsT=wf[:dp, ci, :], rhs=ycT[ci][:dp, :],
                          start=(ci == 0), stop=(ci == len(DCH) - 1))
            act.activation(out=hT[fi], in_=ps, func=Act.Gelu_apprx_tanh,
                           bias=bfc_col[:, f:f + 1], scale=1.0)
        for j0, (c0, cw) in enumerate(NJ):
            wos = []
            for fi in range(ng):
                f = f_base + fi
                wo = pool_w2.tile([128, 400], bf16, name=f"wo{f}_{j0}",
                                  tag="w400", bufs=2 * max(GRP))
                sync.dma_start(out=wo, in_=bass.AP(
                    tensor=wout.tensor, offset=f * 128 * D + c0,
                    ap=[[D, 128], [1, cw]]))
                wos.append(wo)
            for tt in range(4):
                ops = psC.tile([128, 400], f32, name="ops", tag="psc1")
                for fi in range(ng):
                    te.matmul(ops, lhsT=hT[fi][:, tt * 128:(tt + 1) * 128],
                              rhs=wos[fi], start=(fi == 0), stop=(fi == ng - 1))
                vec.tensor_tensor(out=y[tt][:, c0:c0 + cw],
                                  in0=y[tt][:, c0:c0 + cw], in1=ops,
                                  op=AluOp.add)
                if last_grp:
                    # fold the final bias add + store into the last group's
                    # tail so the output drains per 400-col block
                    vec.tensor_tensor(out=y[tt][:, c0:c0 + cw],
                                      in0=y[tt][:, c0:c0 + cw],
                                      in1=bout_rep[:, c0:c0 + cw],
                                      op=AluOp.add)
                    sync.dma_start(out=out[tt * 128:(tt + 1) * 128,
                                           c0:c0 + cw],
                                   in_=y[tt][:, c0:c0 + cw])
        f_base += ng

    pool_w2.release()
    pool_h.release()
    pool_s4.release()
    pool_y.release()
    persist.release()
    psD.release()
    psC.release()


_cached_nc = None


def _get_nc():
    global _cached_nc
    if _cached_nc is None:
        _cached_nc = _build()
    return _cached_nc


NEG = np.float32(-1e10)


def _host_trimask():
    """Within-chunk causal triangle, additive (0 valid / -1e10 future)."""
    l_idx = np.arange(LC)[:, None]
    q_idx = np.arange(CH)[None, :]
    m = np.empty((128, 2, CH), np.float32)
    m[:, 0, :] = np.where(l_idx <= q_idx, np.float32(0), NEG)
    m[:, 1, :] = np.where(l_idx + LC <= q_idx, np.float32(0), NEG)
    return m.astype(BF16NP)


def host_inputs(x, g1, b1, w_qkv, bias_qkv, w_proj, bias_proj, g2, b2, w_fc,
                bias_fc, w_out, bias_out):
    """Per-core in_maps for the SPMD launch (shared + per-core tensors)."""
    x = np.asarray(x, np.float32)
    xf = x.reshape(B * S, D)

    # fold LN1 affine into qkv weights; pre-scale q by c^-0.5
    wqkv_m = (np.asarray(w_qkv) * np.asarray(g1)[:, None]).astype(np.float32)
    bqkv_m = (np.asarray(bias_qkv) + np.asarray(b1) @ np.asarray(w_qkv)).astype(
        np.float32)
    sc = 1.0 / np.sqrt(C)
    wqkv_m[:, :D] *= sc
    bqkv_m[:D] *= sc
    wfc_m = (np.asarray(w_fc) * np.asarray(g2)[:, None]).astype(np.float32)
    bfc_m = (np.asarray(bias_fc) + np.asarray(b2) @ np.asarray(w_fc)).astype(
        np.float32)

    common = {
        "wqkv": np.ascontiguousarray(wqkv_m.astype(BF16NP)),
        "bqkv": np.ascontiguousarray(bqkv_m),
        "wproj": np.ascontiguousarray(np.asarray(w_proj, np.float32).astype(BF16NP)),
        "bproj": np.ascontiguousarray(np.asarray(bias_proj, np.float32)),
        "wfc": np.ascontiguousarray(wfc_m.astype(BF16NP)),
        "bfc": np.ascontiguousarray(bfc_m),
        "wout": np.ascontiguousarray(np.asarray(w_out, np.float32).astype(BF16NP)),
        "bout": np.ascontiguousarray(np.asarray(bias_out, np.float32)),
        "trimask": _host_trimask(),
    }
    in_maps = []
    for j in range(N_CORES):
        a0 = CH * j
        b0 = S + CH * (7 - j)
        xl = np.concatenate([xf[a0:a0 + CH], xf[b0:b0 + CH]], axis=0)
        fl = np.zeros((1, 2 * NDS), np.int32)
        fl[0, 0:j] = 1                      # chunk A: slots s < j
        fl[0, NDS:NDS + (7 - j)] = 1        # chunk B: slots s < 7-j
        in_maps.append({
            "x": np.ascontiguousarray(xl),
            "flags": fl,
            **common,
        })
    return in_maps


def kernel(x, g1, b1, w_qkv, bias_qkv, w_proj, bias_proj, g2, b2, w_fc,
           bias_fc, w_out, bias_out):
    in_maps = host_inputs(x, g1, b1, w_qkv, bias_qkv, w_proj, bias_proj,
                          g2, b2, w_fc, bias_fc, w_out, bias_out)
    nc = _get_nc()
    res = run_bass_kernel_spmd(nc, in_maps, core_ids=list(range(N_CORES)))

    of = np.empty((B * S, D), np.float32)
    for j in range(N_CORES):
        o = res.results[j]["out"]
        a0 = CH * j
        b0 = S + CH * (7 - j)
        of[a0:a0 + CH] = o[:CH]
        of[b0:b0 + CH] = o[CH:]
    return of.reshape(B, S, D)
